# revision 23
# baseline (speedup 1.0000x reference)
"""BFP (block floating point) activation quantization kernel for Trainium2.

Problem: x [32, 256, 56, 56] f32; per (batch, 32-channel block, h, w) the 32
channels share an exponent e = floor(log2(max |x|)); quantize each value to
q * 2^(e-2) with q = clip(round(x / 2^(e-2)), -7, 7)  (mantissa=3 bits).

Strategy (pure data-parallel over batch, 4 images per core on 8 cores):
  - One image per SBUF tile laid out as [128p, 32ci, 196j] where partition
    p = 8*h + a encodes (hw-outer h, channel-block a) and the free dim holds
    (channel-within-block ci, hw-inner j), with hw = 196*h + j.  The block-
    inner partition order makes (block, channel) contiguous in the DMA
    iteration, so each image loads/stores with a single 3-dim dma_start
    (the AP balancer merges block and channel into one 256-count dim).
    Spanning all 128 partitions per DMA engages all 16 SDMA engines
    (narrow-partition DMAs measured at half the engine pool).
  - Block max = per-ci-halving |x|/max tree along the free dim (ScalarE
    computes |x|; a strided tensor_reduce measured 1.7x slower than the
    contiguous tree).
  - scale = 2^(e-2) and rscale = 2^(2-e) are derived with int32 bit ops on
    the exponent field (exact powers of two; bitwise and arith ALU ops are
    class-checked and cannot share a dual-op tensor_scalar).
  - v = x * rscale (VectorE, stride-0 broadcast of rscale over ci)
  - round-to-nearest-even via +1.5*2^23 on ScalarE (fused fp32 affine),
    clip in the shifted domain with one dual-op tensor_scalar
    (min C+7, max C-7), then -C on ScalarE.
  - out = q * scale on GpSimd, written as bf16 (the quantized values carry
    at most 4 significant bits, so bf16 is exact) to halve store traffic;
    the host upconverts to f32.
"""

import numpy as np

import concourse.bass as bass
import concourse.tile as tile
from concourse import bacc, mybir
from concourse.bass_utils import run_bass_kernel_spmd

F32 = mybir.dt.float32
BF16 = mybir.dt.bfloat16
I32 = mybir.dt.int32

N_CORES = 8
B, C, H, W = 32, 256, 56, 56
HW = H * W            # 3136
BPC = B // N_CORES    # 4 images per core
NBLK = C // 32        # 8 channel blocks
NH = 16               # hw-outer chunks (16 * 8 blocks = 128 partitions)
J = HW // NH          # 196 hw-inner elements -> 784B load rows
MAGIC = 12582912.0    # 1.5 * 2**23: RNE round-to-integer magic for |v| < 2**22
QMAX = 7.0            # 2**mantissa - 1

_CACHE = {}


def _build_program():
    if "nc" in _CACHE:
        return _CACHE["nc"]
    nc = bacc.Bacc(
        "TRN2",
        target_bir_lowering=False,
        debug=False,
        enable_asserts=False,
        num_devices=N_CORES,
    )
    x = nc.dram_tensor("x", [BPC, C, HW], F32, kind="ExternalInput")
    # Output in bf16: quantized values are exactly representable; host
    # upconverts to f32.
    y = nc.dram_tensor("y", [BPC, C, HW], BF16, kind="ExternalOutput")

    with tile.TileContext(nc) as tc:
        with (
            tc.tile_pool(name="consts", bufs=1) as cpool,
            tc.tile_pool(name="xp", bufs=2) as xp,
            tc.tile_pool(name="wp", bufs=3) as wp,
            tc.tile_pool(name="op", bufs=2) as op_,
            tc.tile_pool(name="mp", bufs=2) as mp,
        ):
            bias_p = cpool.tile([128, 1], F32, tag="bias_p")
            nc.vector.memset(bias_p[:], MAGIC)
            bias_n = cpool.tile([128, 1], F32, tag="bias_n")
            nc.vector.memset(bias_n[:], -MAGIC)

            for img in range(BPC):
                xt = xp.tile([128, 32, J], F32)
                # single whole-image load; partition-major iteration is
                # (h, a, ci, j) and (a, ci) merges to one 256-channel dim
                dram_in = bass.AP(
                    x, img * C * HW, [[J, NH], [HW, C], [1, J]],
                )
                nc.sync.dma_start(xt[:], dram_in)

                # maxabs over the 32 channels: |x| on ScalarE (spare
                # capacity), then an in-place contiguous max tree on VectorE
                aa = wp.tile([128, 32, J], F32, tag="w")
                nc.scalar.activation(
                    aa[:], xt[:], mybir.ActivationFunctionType.Abs,
                )
                for wdt in (16, 8, 4, 2, 1):
                    nc.vector.tensor_tensor(
                        out=aa[:, 0:wdt, :],
                        in0=aa[:, 0:wdt, :], in1=aa[:, wdt : 2 * wdt, :],
                        op=mybir.AluOpType.max,
                    )
                # scale_bits = (bits(maxabs) & 0x7F800000) - (2 << 23)
                # rscale_bits = 0x7F000000 - scale_bits == NOT(s) + 0x7F000001
                sc = mp.tile([128, J], F32, tag="sc")
                rs = mp.tile([128, J], F32, tag="rs")
                nc.vector.tensor_scalar(
                    out=sc[:].bitcast(I32), in0=aa[:, 0, :].bitcast(I32),
                    scalar1=0x7F800000, scalar2=None,
                    op0=mybir.AluOpType.bitwise_and,
                )
                nc.vector.tensor_scalar(
                    out=sc[:].bitcast(I32), in0=sc[:].bitcast(I32),
                    scalar1=0x01000000, scalar2=None,
                    op0=mybir.AluOpType.subtract,
                )
                nc.vector.tensor_scalar(
                    out=rs[:].bitcast(I32), in0=sc[:].bitcast(I32),
                    scalar1=-1, scalar2=None,
                    op0=mybir.AluOpType.bitwise_xor,
                )
                nc.vector.tensor_scalar(
                    out=rs[:].bitcast(I32), in0=rs[:].bitcast(I32),
                    scalar1=0x7F000001, scalar2=None,
                    op0=mybir.AluOpType.add,
                )

                rsb = rs[:].unsqueeze(1).broadcast_to([128, 32, J])
                scb = sc[:].unsqueeze(1).broadcast_to([128, 32, J])

                # v = x * rscale
                v = wp.tile([128, 32, J], F32, tag="w")
                nc.vector.tensor_tensor(
                    out=v[:], in0=xt[:], in1=rsb, op=mybir.AluOpType.mult,
                )
                # r1 = v + 1.5*2^23 (RNE round in the fp32 affine)
                nc.scalar.activation(
                    v[:], v[:], mybir.ActivationFunctionType.Identity,
                    bias=bias_p[:], scale=1.0,
                )
                # clip in the shifted domain: min(r1, C+7), max(r1, C-7)
                nc.vector.tensor_scalar(
                    out=v[:], in0=v[:],
                    scalar1=MAGIC + QMAX, scalar2=MAGIC - QMAX,
                    op0=mybir.AluOpType.min, op1=mybir.AluOpType.max,
                )
                # undo the magic: q = clipped - C
                nc.scalar.activation(
                    v[:], v[:], mybir.ActivationFunctionType.Identity,
                    bias=bias_n[:], scale=1.0,
                )
                # out = q * scale on GpSimd, bf16
                ot = op_.tile([128, 32, J], BF16, tag="ot")
                nc.gpsimd.tensor_tensor(
                    out=ot[:], in0=v[:], in1=scb, op=mybir.AluOpType.mult,
                )

                # single whole-image store
                dram_out = bass.AP(
                    y, img * C * HW, [[J, NH], [HW, C], [1, J]],
                )
                nc.sync.dma_start(dram_out, ot[:])

    nc.compile()
    _CACHE["nc"] = nc
    return nc


def kernel(activations=None, mantissa=3, blk=32, **_unused):
    x = np.ascontiguousarray(np.asarray(activations), dtype=np.float32)
    assert x.shape == (B, C, H, W), x.shape
    assert int(mantissa) == 3 and int(blk) == 32, (mantissa, blk)

    nc = _build_program()
    xr = x.reshape(B, C, HW)
    in_maps = [{"x": xr[c * BPC : (c + 1) * BPC]} for c in range(N_CORES)]
    res = run_bass_kernel_spmd(nc, in_maps, list(range(N_CORES))).results
    out = np.concatenate(
        [np.asarray(res[c]["y"]).astype(np.float32).reshape(BPC, C, H, W)
         for c in range(N_CORES)],
        axis=0,
    )
    return out


def run_traced(activations):
    """test.py helper: run with NTFF tracing, return (out, BassKernelResults)."""
    x = np.ascontiguousarray(np.asarray(activations), dtype=np.float32)
    nc = _build_program()
    xr = x.reshape(B, C, HW)
    in_maps = [{"x": xr[c * BPC : (c + 1) * BPC]} for c in range(N_CORES)]
    r = run_bass_kernel_spmd(nc, in_maps, list(range(N_CORES)), trace=True)
    out = np.concatenate(
        [np.asarray(r.results[c]["y"]).astype(np.float32).reshape(BPC, C, H, W)
         for c in range(N_CORES)],
        axis=0,
    )
    return out, r
